# revision 1
# baseline (speedup 1.0000x reference)
# Bass/Tile TRN2 kernel for nn_BiasedCrossDecoderLayer (dense cross-attention
# transformer decoder layer), SPMD over 8 NeuronCores.
#
# Sharding: core c -> batch b = c//4, head-group hg = c%4 (4 of 16 heads =
# 256 of 1024 qkv feature dims).  Attention is head-parallel; the
# out-projection produces partial sums which are ReduceScattered (along the
# query axis) within each 4-core batch group; the FFN then runs
# sequence-parallel on each core's 256-query slice with the full 4096 hidden.
#
# LayerNorms are folded into the weights host-side:
#   q = LN(x;g,b) @ Wq.T + pq  ==  LN0(x) @ Wq'.T + bias'
#   with Wq' = wq*g (and the 1/sqrt(hd) attention scale folded into Wq'/bias'),
#   bias' = wq@b + pq, and LN0 the pure normalize (x - m) / std.
# In feature-major layout (activations stored transposed, [feature, token]):
#   qT = rB * (Wq' @ xT_raw + ADJ)
#   ADJ[o,t] = -rowsum(Wq')[o]*m[t] + bias'[o]*std[t]    (rank-2, emitted as a
#              K=2 matmul appended to the same PSUM accumulation group)
# so each projection needs exactly one DVE pass.  V is produced token-major
# (the PV matmul needs it) with the analogous fold.  No on-chip transposes.
#
# Attention runs in the transposed [s, q] layout (mask is pre-transposed on
# the host).  The softmax denominator comes from a ones-column appended to
# the V stationary operand (M=65 matmul); 1/sum is applied after the PV
# accumulation via a K=1 broadcast matmul + one DVE multiply.
#
# All matmuls read operands as float32r (fp32 bits at ~fp22 precision, full
# PE rate; plain fp32 would be 4x slower).

import os
import sys

import numpy as np

sys.path.insert(0, "/opt/trn_rl_repo")

import concourse.bass as bass  # noqa: E402
import concourse.mybir as mybir  # noqa: E402
import concourse.tile as tile  # noqa: E402
from concourse import bacc  # noqa: E402

F32 = mybir.dt.float32
F32R = mybir.dt.float32r
AF = mybir.ActivationFunctionType
ALU = mybir.AluOpType

B, Q, S, D, H = 2, 1024, 2048, 1024, 16
HD = D // H       # 64
FF = 4 * D
EPS = 1e-5
NCORES = 8
NH = 4            # heads per core
FC = NH * HD      # 256 qkv feature dims per core
QS = Q // 4       # 256-query slice per core after reduce-scatter
P = 128
KX = D // P       # 8 k-tiles over the model dim

REPLICA_GROUPS = [[0, 1, 2, 3], [4, 5, 6, 7]]

LAST_RESULT = None  # BassKernelResults of the most recent run (for test.py)


def _r(ap):
    """View an fp32 AP as float32r for full-rate PE matmuls."""
    return ap.bitcast(F32R)


def build_nc():
    nc = bacc.Bacc(
        "TRN2",
        target_bir_lowering=False,
        debug=False,
        num_devices=NCORES,
        name="biased_cross_decoder",
    )

    # ---- DRAM I/O (per-core shards; same program on all cores) ----
    d = {}
    d["ones_t"] = nc.dram_tensor("ones_t", [P, P], F32R, kind="ExternalInput").ap()
    d["xT"] = nc.dram_tensor("xT", [D, Q], F32R, kind="ExternalInput").ap()
    d["zT"] = nc.dram_tensor("zT", [D, S], F32R, kind="ExternalInput").ap()
    d["maskT"] = nc.dram_tensor("maskT", [NH, S, Q], F32, kind="ExternalInput").ap()
    d["wqT"] = nc.dram_tensor("wqT", [P, KX, FC], F32R, kind="ExternalInput").ap()
    d["wkT"] = nc.dram_tensor("wkT", [P, KX, FC], F32R, kind="ExternalInput").ap()
    d["wvT"] = nc.dram_tensor("wvT", [P, KX, FC], F32R, kind="ExternalInput").ap()
    d["adjq"] = nc.dram_tensor("adjq", [2, FC], F32R, kind="ExternalInput").ap()
    d["adjk"] = nc.dram_tensor("adjk", [2, FC], F32R, kind="ExternalInput").ap()
    d["adjv"] = nc.dram_tensor("adjv", [2, FC], F32R, kind="ExternalInput").ap()
    d["owh"] = nc.dram_tensor("owh", [HD, NH, D], F32R, kind="ExternalInput").ap()
    d["outb"] = nc.dram_tensor("outb", [D], F32, kind="ExternalInput").ap()
    d["xq"] = nc.dram_tensor("xq", [D, QS], F32, kind="ExternalInput").ap()
    d["w1p"] = nc.dram_tensor("w1p", [FF // P, P, KX, P], F32R, kind="ExternalInput").ap()
    d["adjf"] = nc.dram_tensor("adjf", [2, FF], F32R, kind="ExternalInput").ap()
    d["w2T"] = nc.dram_tensor("w2T", [FF, D], F32R, kind="ExternalInput").ap()
    d["b2"] = nc.dram_tensor("b2", [D], F32, kind="ExternalInput").ap()
    d["out"] = nc.dram_tensor("out", [D, QS], F32, kind="ExternalOutput").ap()

    with tile.TileContext(nc) as tc:
        build_tile_program(tc, nc, d)
    nc.compile()   # bacc passes: wait splitting, ldweights fusion, reg alloc
    return nc


class _Pool:
    """Keeps the tile_pool context manager alive; allows explicit close."""

    def __init__(self, cm):
        self._cm = cm
        self.pool = cm.__enter__()

    def tile(self, *a, **kw):
        kw.setdefault("name", kw.get("tag") or "t")
        return self.pool.tile(*a, **kw)

    def close(self):
        self._cm.__exit__(None, None, None)


def build_tile_program(tc, nc, d):
    # ---------------- persistent constants ----------------
    const = _Pool(tc.tile_pool(name="const", bufs=1))
    dram = _Pool(tc.tile_pool(name="dram", bufs=1, space="DRAM"))

    # ones come from DRAM: DVE memset cannot write fp32r (ISA limitation)
    ones_sb = const.tile([P, P], F32R, tag="ones_sb")
    nc.sync.dma_start(ones_sb, d["ones_t"])
    ones_col = ones_sb[:, 0:1]                  # lhsT for column sums
    ones_row = ones_sb[0:1, :]                  # lhsT for partition broadcasts
    ones65 = ones_sb                            # row 64 used as base-64 lhsT
    eps_t = const.tile([1, 1], F32, tag="eps")
    nc.vector.memset(eps_t, EPS)
    outb_col = const.tile([P, KX], F32, tag="outb_col")
    nc.sync.dma_start(outb_col, d["outb"].rearrange("(o p) -> p o", p=P))
    b2_col = const.tile([P, KX], F32, tag="b2_col")
    nc.sync.dma_start(b2_col, d["b2"].rearrange("(o p) -> p o", p=P))
    rz_col = const.tile([P, S // P], F32R, tag="rz_col")    # rstd_z token-striped

    rs_scr = dram.tile([1, S], F32R, tag="rs_scr")          # row restripe bounce

    # ---------------- long-lived right-side pools ----------------
    # (separate allocator stack: released in LIFO order attp after qkv)
    pool_att = _Pool(tc.tile_pool(name="attp", bufs=1, side="right"))
    pool_qkv = _Pool(tc.tile_pool(name="qkv", bufs=1, side="right"))

    # ---------------- phase A/B scratch pools (left stack) ----------------
    pool_rows = _Pool(tc.tile_pool(name="rows", bufs=2))
    pool_sq = _Pool(tc.tile_pool(name="sq", bufs=3))
    pool_adj = _Pool(tc.tile_pool(name="adj", bufs=1))
    pool_z = _Pool(tc.tile_pool(name="pz", bufs=1))
    pool_x = _Pool(tc.tile_pool(name="px", bufs=1))

    # stat row tiles ([2, T] lhsT/rhs operands for the rank-2 ADJ matmuls)
    adjx = pool_adj.tile([2, Q], F32R, tag="adjx")          # [mx ; stdx]
    adjz = pool_adj.tile([2, S], F32R, tag="adjz")          # [mz ; stdz]

    xT = pool_x.tile([P, KX, Q], F32R, tag="xT")
    for ch in range(2):
        for k in range(KX):
            nc.sync.dma_start(xT[:, k, ch * 512:(ch + 1) * 512],
                              d["xT"][k * P:(k + 1) * P, ch * 512:(ch + 1) * 512])
    zT = pool_z.tile([P, KX, S], F32R, tag="zT")
    for ch in range(4):
        for k in range(KX):
            nc.sync.dma_start(zT[:, k, ch * 512:(ch + 1) * 512],
                              d["zT"][k * P:(k + 1) * P, ch * 512:(ch + 1) * 512])

    def ln_stats2(aT, T, adj, rB, ps_stats, ps_bcp, scr=None):
        """Per 512-token chunk: LN stats -> adj=[mean;std] rows, broadcast
        1/std into rB [P,T]; optionally stash 1/std to scr (DRAM)."""
        for ch in range(T // 512):
            sl = slice(ch * 512, (ch + 1) * 512)
            ps_sum = ps_stats.tile([1, 512], F32, name="ps_sum", tag="ps_sum")
            ps_ssq = ps_stats.tile([1, 512], F32, name="ps_ssq", tag="ps_ssq")
            for k in range(KX):
                nc.tensor.matmul(ps_sum, _r(ones_col), _r(aT[:, k, sl]),
                                 start=(k == 0), stop=(k == KX - 1))
                sq = pool_sq.tile([P, 512], F32R, name="sq", tag="sq")
                nc.scalar.square(sq, aT[:, k, sl])
                nc.tensor.matmul(ps_ssq, _r(ones_col), _r(sq),
                                 start=(k == 0), stop=(k == KX - 1))
            e2 = pool_rows.tile([1, 512], F32, name="e2", tag="e2")
            m2 = pool_rows.tile([1, 512], F32, name="m2", tag="m2")
            inv = pool_rows.tile([1, 512], F32R, name="inv", tag="inv")
            rr = pool_rows.tile([1, 512], F32R, name="rr", tag="rr")
            nc.vector.tensor_scalar_mul(adj[0:1, sl], ps_sum, 1.0 / D)  # mean
            nc.vector.tensor_scalar_mul(e2, ps_ssq, 1.0 / D)            # E[x^2]
            nc.vector.tensor_mul(m2, adj[0:1, sl], adj[0:1, sl])
            nc.vector.tensor_sub(e2, e2, m2)                            # var
            nc.scalar.activation(inv, e2, AF.Sqrt, bias=eps_t[0:1])     # std
            with nc.allow_low_precision(reason="fp32r rounding of 1/std"):
                nc.vector.reciprocal(rr, inv)
            nc.sync.dma_start(adj[1:2, sl], inv)   # cross-partition row move
            bc = ps_bcp.tile([P, 512], F32, name="bc", tag="bc")
            nc.tensor.matmul(bc, _r(ones_row), _r(rr))
            nc.scalar.copy(rB[:, sl], bc)
            if scr is not None:
                nc.sync.dma_start(scr[0:1, sl], rr)

    # ---- x statistics + broadcast of rx ----
    pool_bcx = _Pool(tc.tile_pool(name="bcx", bufs=1))
    rxB = pool_bcx.tile([P, Q], F32, tag="rxB")
    with tc.tile_pool(name="ps_sx", bufs=2, space="PSUM") as ps_sx, \
         tc.tile_pool(name="ps_bcx", bufs=2, space="PSUM") as ps_bcx:
        ln_stats2(xT, Q, adjx, rxB, ps_sx, ps_bcx)

    # ---- q projection (feature-major) ----
    qT = pool_qkv.tile([P, FC // P, Q], F32R, tag="qT")   # includes 1/8 scale
    kT = pool_qkv.tile([P, FC // P, S], F32R, tag="kT")
    v_sb = pool_qkv.tile([P, S // P, NH, HD + 1], F32R, tag="v_sb")

    with tc.tile_pool(name="wq", bufs=1) as pool_wq, \
         tc.tile_pool(name="ps_q", bufs=3, space="PSUM") as ps_qk:
        wq_sb = pool_wq.tile([P, KX, FC], F32R, tag="wq_sb")
        nc.sync.dma_start(wq_sb, d["wqT"])
        adjq_w = pool_wq.tile([2, FC], F32R, tag="adjq_w")
        nc.sync.dma_start(adjq_w, d["adjq"])
        for m in range(FC // P):
            for ch in range(Q // 512):
                sl = slice(ch * 512, (ch + 1) * 512)
                ps = ps_qk.tile([P, 512], F32, name="ps_qk_t", tag="ps_qk_t")
                for k in range(KX):
                    nc.tensor.matmul(ps, _r(wq_sb[:, k, m * P:(m + 1) * P]),
                                     _r(xT[:, k, sl]), start=(k == 0), stop=False)
                nc.tensor.matmul(ps, _r(adjq_w[:, m * P:(m + 1) * P]),
                                 _r(adjx[:, sl]), start=False, stop=True)
                nc.vector.tensor_mul(qT[:, m, sl], ps, rxB[:, sl])

    pool_bcx.close()
    pool_x.close()

    # ---- z statistics + broadcast of rz + restripe rz to columns ----
    pool_bcz = _Pool(tc.tile_pool(name="bcz", bufs=1))
    rzB = pool_bcz.tile([P, S], F32, tag="rzB")
    with tc.tile_pool(name="ps_sz", bufs=2, space="PSUM") as ps_sz, \
         tc.tile_pool(name="ps_bcz", bufs=2, space="PSUM") as ps_bcz:
        ln_stats2(zT, S, adjz, rzB, ps_sz, ps_bcz, scr=rs_scr)
    nc.sync.dma_start(rz_col, rs_scr.rearrange("a (i p) -> (a p) i", p=P))

    # ---- k / v projections ----
    # softmax-denominator ones column (DMA: DVE memset cannot write fp32r)
    nc.sync.dma_start(
        v_sb[:, :, :, HD:HD + 1],
        d["ones_t"][:, 0:S // P * NH].rearrange("p (a b c) -> p a b c",
                                                a=S // P, c=1))

    with tc.tile_pool(name="wkv", bufs=1) as pool_wkv, \
         tc.tile_pool(name="ps_k", bufs=3, space="PSUM") as ps_qk, \
         tc.tile_pool(name="ps_v", bufs=2, space="PSUM") as ps_v:
        wk_sb = pool_wkv.tile([P, KX, FC], F32R, tag="wk_sb")
        nc.sync.dma_start(wk_sb, d["wkT"])
        wv_sb = pool_wkv.tile([P, KX, FC], F32R, tag="wv_sb")
        nc.sync.dma_start(wv_sb, d["wvT"])
        adjk_w = pool_wkv.tile([2, FC], F32R, tag="adjk_w")
        nc.sync.dma_start(adjk_w, d["adjk"])
        adjv_w = pool_wkv.tile([2, FC], F32R, tag="adjv_w")
        nc.sync.dma_start(adjv_w, d["adjv"])

        for m in range(FC // P):
            for ch in range(S // 512):
                sl = slice(ch * 512, (ch + 1) * 512)
                ps = ps_qk.tile([P, 512], F32, name="ps_qk_t", tag="ps_qk_t")
                for k in range(KX):
                    nc.tensor.matmul(ps, _r(wk_sb[:, k, m * P:(m + 1) * P]),
                                     _r(zT[:, k, sl]), start=(k == 0), stop=False)
                nc.tensor.matmul(ps, _r(adjk_w[:, m * P:(m + 1) * P]),
                                 _r(adjz[:, sl]), start=False, stop=True)
                nc.vector.tensor_mul(kT[:, m, sl], ps, rzB[:, sl])

        # v: token-major; ADJ lhsT = [mz ; stdz], rhs = [-rowsum(Wv') ; biasv']
        for t in range(S // P):
            ps = ps_v.tile([P, FC], F32, name="ps_v_t", tag="ps_v_t")
            for k in range(KX):
                nc.tensor.matmul(ps, _r(zT[:, k, t * P:(t + 1) * P]),
                                 _r(wv_sb[:, k, :]), start=(k == 0), stop=False)
            nc.tensor.matmul(ps, _r(adjz[:, t * P:(t + 1) * P]), _r(adjv_w),
                             start=False, stop=True)
            nc.vector.tensor_scalar_mul(
                v_sb[:, t, :, 0:HD],
                ps.rearrange("p (h e) -> p h e", h=NH),
                rz_col[:, t:t + 1].bitcast(F32))

    pool_bcz.close()
    pool_z.close()
    pool_adj.close()
    pool_sq.close()
    pool_rows.close()

    # =================== attention ===================
    att64 = [pool_att.tile([HD, Q], F32R, name=f"att64_{h}", tag=f"att64_{h}")
             for h in range(NH)]

    with tc.tile_pool(name="mask", bufs=16) as pool_mask, \
         tc.tile_pool(name="probs", bufs=5) as pool_probs, \
         tc.tile_pool(name="nbc", bufs=2) as pool_nbc, \
         tc.tile_pool(name="rrow", bufs=2) as pool_rrow, \
         tc.tile_pool(name="ps_lg", bufs=2, space="PSUM") as ps_lg, \
         tc.tile_pool(name="ps_att", bufs=1, space="PSUM") as ps_att, \
         tc.tile_pool(name="ps_nbc", bufs=1, space="PSUM") as ps_nbc:

        for h in range(NH):
            ht, ho = h // 2, HD * (h % 2)
            att_ps = ps_att.tile([HD + 1, Q], F32, name="att_ps", tag="att_ps")
            for st in range(S // P):
                mk = pool_mask.tile([P, Q], F32, name="mk", tag="mk")
                nc.sync.dma_start(mk, d["maskT"][h, st * P:(st + 1) * P, :])
                pr = pool_probs.tile([P, Q], F32R, name="pr", tag="pr")
                lg = ps_lg.tile([P, Q], F32, name="lg", tag="lg")
                for ch in range(Q // 512):
                    sl = slice(ch * 512, (ch + 1) * 512)
                    nc.tensor.matmul(
                        lg[:, sl],
                        _r(kT[ho:ho + HD, ht, st * P:(st + 1) * P]),
                        _r(qT[ho:ho + HD, ht, sl]))
                nc.vector.tensor_add(pr, lg, mk)
                nc.scalar.activation(pr, pr, AF.Exp)
                for ch in range(Q // 512):
                    sl = slice(ch * 512, (ch + 1) * 512)
                    nc.tensor.matmul(att_ps[:, sl], _r(v_sb[:, st, h, :]),
                                     _r(pr[:, sl]),
                                     start=(st == 0), stop=(st == S // P - 1))
            # normalize: att[0:64] * broadcast(1 / att[64])
            rr = pool_rrow.tile([HD + 1, Q], F32R, name="rr", tag="rr")
            with nc.allow_low_precision(reason="fp32r rounding of 1/sum"):
                nc.vector.reciprocal(rr[HD:HD + 1, :], att_ps[HD:HD + 1, :])
            nbc = pool_nbc.tile([HD, Q], F32, name="nbc_t", tag="nbc_t")
            for ch in range(Q // 512):
                sl = slice(ch * 512, (ch + 1) * 512)
                bc = ps_nbc.tile([HD, 512], F32, name="bc2", tag="bc2")
                nc.tensor.matmul(bc, _r(ones65[HD:HD + 1, 0:HD]),
                                 _r(rr[HD:HD + 1, sl]))
                nc.scalar.copy(nbc[:, sl], bc)
            nc.vector.tensor_mul(att64[h], att_ps[0:HD, :], nbc)

    pool_qkv.close()

    # =================== out-projection + ReduceScatter ===================
    DH = D // 2
    rs_in = [dram.tile([4, DH, QS], F32, name=f"rs_in{i}", tag=f"rs_in{i}")
             for i in range(2)]
    rs_out = [dram.tile([DH, QS], F32, name=f"rs_out{i}", tag=f"rs_out{i}")
              for i in range(2)]

    with tc.tile_pool(name="ow", bufs=1) as pool_ow, \
         tc.tile_pool(name="osb", bufs=3) as pool_osb, \
         tc.tile_pool(name="ps_o", bufs=3, space="PSUM") as ps_o:
        ow_sb = pool_ow.tile([HD, NH, D], F32R, tag="ow_sb")
        nc.sync.dma_start(ow_sb, d["owh"])
        for half in range(2):
            for mi in range(D // P // 2):
                m = half * (D // P // 2) + mi
                for ch in range(Q // 512):
                    sl = slice(ch * 512, (ch + 1) * 512)
                    ps = ps_o.tile([P, 512], F32, name="ps_o_t", tag="ps_o_t")
                    for h in range(NH):
                        nc.tensor.matmul(ps, _r(ow_sb[:, h, m * P:(m + 1) * P]),
                                         _r(att64[h][:, sl]),
                                         start=(h == 0), stop=(h == NH - 1))
                    ot = pool_osb.tile([P, 512], F32, name="ot", tag="ot")
                    nc.scalar.copy(ot, ps)
                    for r2 in range(2):
                        nc.sync.dma_start(
                            rs_in[half][2 * ch + r2, mi * P:(mi + 1) * P, :],
                            ot[:, r2 * QS:(r2 + 1) * QS])
            # launch this half's reduce-scatter while the other half computes
            nc.gpsimd.collective_compute(
                "ReduceScatter",
                ALU.add,
                replica_groups=REPLICA_GROUPS,
                ins=[rs_in[half].opt()],
                outs=[rs_out[half].opt()],
            )

    pool_att.close()

    # =================== residual + FFN (sequence-parallel) ===================
    with tc.tile_pool(name="ffn", bufs=1) as pool_f, \
         tc.tile_pool(name="w1s", bufs=8) as pool_w1, \
         tc.tile_pool(name="w2s", bufs=8) as pool_w2, \
         tc.tile_pool(name="gact", bufs=3) as pool_g, \
         tc.tile_pool(name="rsld", bufs=3) as pool_rsld, \
         tc.tile_pool(name="yout", bufs=3) as pool_yo, \
         tc.tile_pool(name="ps_f", bufs=2, space="PSUM") as ps_f, \
         tc.tile_pool(name="ps_y2", bufs=1, space="PSUM") as ps_y2:

        y1T = pool_f.tile([P, KX, QS], F32R, tag="y1T")
        adjy = pool_f.tile([2, QS], F32R, tag="adjy")      # [my ; stdy]
        ry_row = pool_f.tile([1, QS], F32R, tag="ry_row")
        ryB = pool_f.tile([P, QS], F32, tag="ryB")
        adjf_w = pool_f.tile([2, FF], F32R, tag="adjf_w")
        nc.sync.dma_start(adjf_w, d["adjf"])

        # y1 = RS(out-proj partials) + x_slice + out_b   (feature-major)
        # gpsimd DMAs: keep the HWDGE queues free for weight prefetch while
        # the collective is still in flight
        for m in range(KX):
            half, mi = m // (KX // 2), m % (KX // 2)
            rst = pool_rsld.tile([P, QS], F32, name="rst", tag="rst")
            nc.gpsimd.dma_start(rst, rs_out[half][mi * P:(mi + 1) * P, :])
            xqt = pool_rsld.tile([P, QS], F32, name="xqt", tag="xqt")
            nc.gpsimd.dma_start(xqt, d["xq"][m * P:(m + 1) * P, :])
            nc.vector.scalar_tensor_tensor(
                out=y1T[:, m, :], in0=rst, scalar=outb_col[:, m:m + 1],
                in1=xqt, op0=ALU.add, op1=ALU.add)

        # y1 LN stats
        with tc.tile_pool(name="ps_yst", bufs=1, space="PSUM") as ps_yst:
            e2_row = pool_f.tile([1, QS], F32, tag="e2y_row")
            m2_row = pool_f.tile([1, QS], F32, tag="m2y_row")
            inv_row = pool_f.tile([1, QS], F32R, tag="invy_row")
            ps_sum = ps_yst.tile([1, QS], F32, name="ps_sum2", tag="ps_sum2")
            ps_ssq = ps_yst.tile([1, QS], F32, name="ps_ssq2", tag="ps_ssq2")
            for k in range(KX):
                nc.tensor.matmul(ps_sum, _r(ones_col), _r(y1T[:, k, :]),
                                 start=(k == 0), stop=(k == KX - 1))
                sq = pool_g.tile([P, QS], F32R, name="ysq", tag="gt")
                nc.scalar.square(sq, y1T[:, k, :])
                nc.tensor.matmul(ps_ssq, _r(ones_col), _r(sq),
                                 start=(k == 0), stop=(k == KX - 1))
            nc.vector.tensor_scalar_mul(adjy[0:1, :], ps_sum, 1.0 / D)
            nc.vector.tensor_scalar_mul(e2_row, ps_ssq, 1.0 / D)
            nc.vector.tensor_mul(m2_row, adjy[0:1, :], adjy[0:1, :])
            nc.vector.tensor_sub(e2_row, e2_row, m2_row)
            nc.scalar.activation(inv_row, e2_row, AF.Sqrt, bias=eps_t[0:1])
            with nc.allow_low_precision(reason="fp32r rounding of 1/std"):
                nc.vector.reciprocal(ry_row, inv_row)
            nc.sync.dma_start(adjy[1:2, :], inv_row)
            bc = ps_f.tile([P, QS], F32, name="bc3", tag="ps_f_t")
            nc.tensor.matmul(bc, _r(ones_row), _r(ry_row))
            nc.scalar.copy(ryB, bc)

        # ff1 + exact gelu into one persistent [P, 32, QS] activation tile
        g_sb = pool_f.tile([P, FF // P, QS], F32R, tag="g_sb")
        for j in range(FF // P):
            w1b = pool_w1.tile([P, KX, P], F32R, name="w1b", tag="w1b")
            nc.sync.dma_start(w1b, d["w1p"][j])
            ps = ps_f.tile([P, QS], F32, name="ps_f_t", tag="ps_f_t")
            for k in range(KX):
                nc.tensor.matmul(ps, _r(w1b[:, k, :]), _r(y1T[:, k, :]),
                                 start=(k == 0), stop=False)
            nc.tensor.matmul(ps, _r(adjf_w[:, j * P:(j + 1) * P]), _r(adjy),
                             start=False, stop=True)
            nc.vector.tensor_mul(g_sb[:, j, :], ps, ryB)   # ff1 = ry*(raw+adj)
            nc.scalar.activation(g_sb[:, j, :], g_sb[:, j, :], AF.Gelu)

        # ff2 in two half-D passes; each output m-tile gets a full psum bank
        for half in range(2):
            y2a = [ps_y2.tile([P, QS], F32, name=f"y2a_{i}", tag=f"y2a_{i}",
                              bufs=1) for i in range(4)]
            for j in range(FF // P):
                w2b = pool_w2.tile([P, D // 2], F32R, name="w2b", tag="w2b")
                nc.sync.dma_start(
                    w2b, d["w2T"][j * P:(j + 1) * P,
                                  half * (D // 2):(half + 1) * (D // 2)])
                for mi in range(4):
                    nc.tensor.matmul(y2a[mi], _r(w2b[:, mi * P:(mi + 1) * P]),
                                     _r(g_sb[:, j, :]),
                                     start=(j == 0), stop=(j == FF // P - 1))
            for mi in range(4):
                m = half * 4 + mi
                yt = pool_yo.tile([P, QS], F32, name="yt", tag="yt")
                nc.vector.scalar_tensor_tensor(
                    out=yt, in0=y2a[mi], scalar=b2_col[:, m:m + 1],
                    in1=y1T[:, m, :], op0=ALU.add, op1=ALU.add)
                nc.sync.dma_start(d["out"][m * P:(m + 1) * P, :], yt)

    const.close()
    dram.close()


def host_prep(inputs):
    """Fold layernorm gains/biases into weights; build the 8 per-core shards."""
    f32 = np.float32
    x = np.asarray(inputs["x"], f32)
    z = np.asarray(inputs["z"], f32)
    mask = np.asarray(inputs["attn_mask"], f32)
    gq = np.asarray(inputs["gq"], np.float64)
    bq = np.asarray(inputs["bq"], np.float64)
    gkv = np.asarray(inputs["gkv"], np.float64)
    bkv = np.asarray(inputs["bkv"], np.float64)
    gff = np.asarray(inputs["gff"], np.float64)
    bff = np.asarray(inputs["bff"], np.float64)
    ipw = np.asarray(inputs["in_proj_w"], np.float64)
    ipb = np.asarray(inputs["in_proj_b"], np.float64)
    out_w = np.asarray(inputs["out_w"], f32)
    out_b = np.asarray(inputs["out_b"], f32)
    w1 = np.asarray(inputs["w1"], np.float64)
    b1 = np.asarray(inputs["b1"], np.float64)
    w2 = np.asarray(inputs["w2"], f32)
    b2 = np.asarray(inputs["b2"], f32)

    wq, wk, wv = ipw[:D], ipw[D:2 * D], ipw[2 * D:]
    pq, pk, pv = ipb[:D], ipb[D:2 * D], ipb[2 * D:]
    scale = 1.0 / np.sqrt(HD)
    wq2 = (wq * gq[None, :]) * scale
    pq2 = (wq @ bq + pq) * scale
    wk2 = wk * gkv[None, :]
    pk2 = wk @ bkv + pk
    wv2 = wv * gkv[None, :]
    pv2 = wv @ bkv + pv
    w12 = w1 * gff[None, :]
    b12 = w1 @ bff + b1

    w1T = np.ascontiguousarray(w12.T.astype(f32))                    # (D, FF)
    # packed so each hidden-block's [P, KX, P] lhsT tile set is contiguous
    w1p = np.ascontiguousarray(
        w1T.reshape(KX, P, FF // P, P).transpose(2, 1, 0, 3))
    adjf = np.ascontiguousarray(
        np.stack([-w12.sum(1), b12]).astype(f32))                    # (2, FF)
    w2T = np.ascontiguousarray(w2.T)                                 # (FF, D)

    def pack_kxf(wT):  # (D, FC) -> (P, D//P, FC)
        return np.ascontiguousarray(wT.reshape(KX, P, FC).transpose(1, 0, 2))

    in_maps = []
    for c in range(NCORES):
        b, hg = c // 4, c % 4
        fs = slice(FC * hg, FC * hg + FC)
        qs = slice(QS * (c % 4), QS * (c % 4) + QS)
        xTb = np.ascontiguousarray(x[b].T)                           # (D, Q)
        in_maps.append({
            "ones_t": np.ones((P, P), f32),
            "xT": xTb,
            "zT": np.ascontiguousarray(z[b].T),
            "maskT": np.ascontiguousarray(
                mask[16 * b + NH * hg:16 * b + NH * hg + NH].transpose(0, 2, 1)),
            "wqT": pack_kxf(np.ascontiguousarray(wq2[fs].T.astype(f32))),
            "wkT": pack_kxf(np.ascontiguousarray(wk2[fs].T.astype(f32))),
            "wvT": pack_kxf(np.ascontiguousarray(wv2[fs].T.astype(f32))),
            "adjq": np.ascontiguousarray(
                np.stack([-wq2[fs].sum(1), pq2[fs]]).astype(f32)),
            "adjk": np.ascontiguousarray(
                np.stack([-wk2[fs].sum(1), pk2[fs]]).astype(f32)),
            "adjv": np.ascontiguousarray(
                np.stack([-wv2[fs].sum(1), pv2[fs]]).astype(f32)),
            "owh": np.ascontiguousarray(
                out_w[:, fs].T.reshape(NH, HD, D).transpose(1, 0, 2)),
            "outb": out_b,
            "xq": np.ascontiguousarray(xTb[:, qs]),
            "w1p": w1p,
            "adjf": adjf,
            "w2T": w2T,
            "b2": b2,
        })
    return in_maps


_NC_CACHE = None


def kernel(**inputs) -> np.ndarray:
    global _NC_CACHE, LAST_RESULT
    from concourse.bass_utils import run_bass_kernel_spmd

    in_maps = host_prep(inputs)
    if _NC_CACHE is None:
        _NC_CACHE = build_nc()
    res = run_bass_kernel_spmd(
        _NC_CACHE, in_maps, core_ids=list(range(NCORES)),
        trace=bool(os.environ.get("BASS_TRACE")),
    )
    LAST_RESULT = res
    out = np.empty((B, Q, D), np.float32)
    for c in range(NCORES):
        b = c // 4
        qs = slice(QS * (c % 4), QS * (c % 4) + QS)
        out[b, qs, :] = res.results[c]["out"].T
    return out



# revision 8
# speedup vs baseline: 1.2105x; 1.2105x over previous
# Bass/Tile TRN2 kernel for nn_BiasedCrossDecoderLayer (dense cross-attention
# transformer decoder layer), SPMD over 8 NeuronCores.
#
# Sharding: core c -> batch b = c//4, head-group hg = c%4 (4 of 16 heads).
# Attention is head-parallel and processed in two query chunks of 512; after
# each chunk the out-projection partial sums are ReduceScattered (bf16) within
# the 4-core batch group so the collective overlaps the next chunk's compute.
# Query ownership is interleaved: core c owns queries [128c,128c+128) of chunk
# A and [512+128c, 512+128c+128) of chunk B (the host gather restores order),
# so each chunk's RS delivers a 128-query piece directly.  The FFN then runs
# sequence-parallel on the core's 256 owned queries with the full 4096 hidden.
#
# LayerNorms are folded into the weights host-side (bf16 weights; rank-2
# [mean;std] correction matmuls in fp32r appended to each PSUM group).  The
# attention mask is added to the logits by an identity-matmul into the same
# PSUM accumulation group (no DVE pass); softmax denominators come from a
# ones-column appended to V (M=65 PV matmul).  The whole heavy datapath is
# bf16 (weights, activations, mask, probs) which halves DMA traffic and
# enables fast weight loads; PSUM accumulation stays fp32.  QK logits for a
# head pair run concurrently in two 64-row PE groups (K=64 row tiling).
#
# The FFN weights (w1 AND w2, bf16) are DMA'd into SBUF during attention so
# ff1/ff2 run back-to-back per hidden block with zero weight stalls.

import os
import sys

import numpy as np

sys.path.insert(0, "/opt/trn_rl_repo")

import ml_dtypes  # noqa: E402

import concourse.bass as bass  # noqa: E402
import concourse.mybir as mybir  # noqa: E402
import concourse.tile as tile  # noqa: E402
from concourse import bacc  # noqa: E402

F32 = mybir.dt.float32
F32R = mybir.dt.float32r
BF16 = mybir.dt.bfloat16
AF = mybir.ActivationFunctionType
ALU = mybir.AluOpType

B, Q, S, D, H = 2, 1024, 2048, 1024, 16
HD = D // H       # 64
FF = 4 * D
EPS = 1e-5
NCORES = 8
NH = 4            # heads per core
FC = NH * HD      # 256 qkv feature dims per core
QS = 256          # queries owned per core (two 128-query pieces)
QC = 512          # attention query chunk
P = 128
KX = D // P       # 8 k-tiles over the model dim
FFP = FF // P     # 32 hidden blocks
NST = S // P      # 16 s-tiles

REPLICA_GROUPS = [[0, 1, 2, 3], [4, 5, 6, 7]]

LAST_RESULT = None  # BassKernelResults of the most recent run (for test.py)


def _r(ap):
    return ap.bitcast(F32R)


def _f(ap):
    return ap.bitcast(F32)


def build_nc():
    nc = bacc.Bacc(
        "TRN2",
        target_bir_lowering=False,
        debug=False,
        num_devices=NCORES,
        name="biased_cross_decoder",
    )

    d = {}
    d["ones_t"] = nc.dram_tensor("ones_t", [P, P], F32R, kind="ExternalInput").ap()
    d["cb"] = nc.dram_tensor("cb", [P, P + 64], BF16, kind="ExternalInput").ap()
    d["xT"] = nc.dram_tensor("xT", [D, Q], BF16, kind="ExternalInput").ap()
    d["zT"] = nc.dram_tensor("zT", [D, S], BF16, kind="ExternalInput").ap()
    d["xq"] = nc.dram_tensor("xq", [D, QS], F32, kind="ExternalInput").ap()
    d["maskT"] = nc.dram_tensor("maskT", [2, NH, S, QC], BF16,
                                kind="ExternalInput").ap()
    d["wqT"] = nc.dram_tensor("wqT", [P, KX, FC], BF16, kind="ExternalInput").ap()
    d["wkT"] = nc.dram_tensor("wkT", [P, KX, FC], BF16, kind="ExternalInput").ap()
    d["wvT"] = nc.dram_tensor("wvT", [P, KX, FC], BF16, kind="ExternalInput").ap()
    d["adjq"] = nc.dram_tensor("adjq", [2, FC], F32R, kind="ExternalInput").ap()
    d["adjk"] = nc.dram_tensor("adjk", [2, FC], F32R, kind="ExternalInput").ap()
    d["adjv"] = nc.dram_tensor("adjv", [2, FC], F32R, kind="ExternalInput").ap()
    d["owp"] = nc.dram_tensor("owp", [P, 2, D], BF16, kind="ExternalInput").ap()
    d["outb"] = nc.dram_tensor("outb", [D], F32, kind="ExternalInput").ap()
    d["b1c"] = nc.dram_tensor("b1c", [P, FFP], F32, kind="ExternalInput").ap()
    d["b2"] = nc.dram_tensor("b2", [D], F32, kind="ExternalInput").ap()
    d["w1p"] = nc.dram_tensor("w1p", [FFP, P, KX, P], BF16,
                              kind="ExternalInput").ap()
    d["w2T"] = nc.dram_tensor("w2T", [FF, D], BF16, kind="ExternalInput").ap()
    d["out"] = nc.dram_tensor("out", [D, QS], F32, kind="ExternalOutput").ap()

    with tile.TileContext(nc) as tc:
        build_tile_program(tc, nc, d)
    nc.compile()
    return nc


class _Pool:
    """Keeps the tile_pool context manager alive; allows explicit close."""

    def __init__(self, cm):
        self._cm = cm
        self.pool = cm.__enter__()

    def tile(self, *a, **kw):
        kw.setdefault("name", kw.get("tag") or "t")
        return self.pool.tile(*a, **kw)

    def close(self):
        self._cm.__exit__(None, None, None)


def build_tile_program(tc, nc, d):
    # ---------------- persistent constants ----------------
    const = _Pool(tc.tile_pool(name="const", bufs=1))
    dram = _Pool(tc.tile_pool(name="dram", bufs=1, space="DRAM"))

    ones_sb = const.tile([P, P], F32R, tag="ones_sb")
    nc.sync.dma_start(ones_sb, d["ones_t"])
    cb_sb = const.tile([P, P + 64], BF16, tag="cb_sb")
    nc.sync.dma_start(cb_sb, d["cb"])
    ident = cb_sb[:, 0:P]            # bf16 identity (mask-add matmul lhsT)
    ones_bcol = cb_sb[:, P:P + 1]    # bf16 ones column (bf16 stat sums)
    ones_col = ones_sb[:, 0:1]       # f32r ones column (fp32 stat sums)
    ones_row = ones_sb[0:1, :]       # f32r row (partition broadcasts)

    eps_t = const.tile([1, 1], F32, tag="eps")
    nc.vector.memset(eps_t, EPS)
    outb_col = const.tile([P, KX], F32, tag="outb_col")
    nc.sync.dma_start(outb_col, d["outb"].rearrange("(o p) -> p o", p=P))
    b2_col = const.tile([P, KX], F32, tag="b2_col")
    nc.sync.dma_start(b2_col, d["b2"].rearrange("(o p) -> p o", p=P))
    b1_col = const.tile([P, FFP], F32, tag="b1_col")
    nc.sync.dma_start(b1_col, d["b1c"])
    xq_sb = const.tile([P, KX, QS], F32, tag="xq_sb")
    for k in range(KX):
        nc.sync.dma_start(xq_sb[:, k, :], d["xq"][k * P:(k + 1) * P, :])
    rz_col = const.tile([P, NST], F32R, tag="rz_col")

    rs_scr = dram.tile([1, S], F32R, tag="rs_scr")
    rs_in = [dram.tile([4, D, P], BF16, name=f"rs_in{i}", tag=f"rs_in{i}")
             for i in range(2)]
    rs_out = [dram.tile([D, P], BF16, name=f"rs_out{i}", tag=f"rs_out{i}")
              for i in range(2)]

    # ---------------- long-lived right-side pools ----------------
    pool_qkv = _Pool(tc.tile_pool(name="qkv", bufs=1, side="right"))
    qT = pool_qkv.tile([P, 2, Q], BF16, tag="qT")
    kT = pool_qkv.tile([P, 2, S], BF16, tag="kT")
    v_sb = pool_qkv.tile([P, NST, NH, HD + 1], BF16, tag="v_sb")
    ow_sb = pool_qkv.tile([P, 2, D], BF16, tag="ow_sb")
    nc.sync.dma_start(ow_sb, d["owp"])

    # ---------------- phase A scratch (left stack) ----------------
    pool_x = _Pool(tc.tile_pool(name="px", bufs=1))
    pool_z = _Pool(tc.tile_pool(name="pz", bufs=1))
    pool_w = _Pool(tc.tile_pool(name="pw", bufs=1))
    pool_adj = _Pool(tc.tile_pool(name="adj", bufs=1))
    pool_bc = _Pool(tc.tile_pool(name="bc", bufs=1))
    pool_sq = _Pool(tc.tile_pool(name="sq", bufs=3))
    pool_rows = _Pool(tc.tile_pool(name="rows", bufs=2))

    xT = pool_x.tile([P, KX, Q], BF16, tag="xT")
    for ch in range(2):
        for k in range(KX):
            nc.sync.dma_start(xT[:, k, ch * 512:(ch + 1) * 512],
                              d["xT"][k * P:(k + 1) * P, ch * 512:(ch + 1) * 512])
    zT = pool_z.tile([P, KX, S], BF16, tag="zT")
    for ch in range(4):
        for k in range(KX):
            nc.sync.dma_start(zT[:, k, ch * 512:(ch + 1) * 512],
                              d["zT"][k * P:(k + 1) * P, ch * 512:(ch + 1) * 512])

    wq_sb = pool_w.tile([P, KX, FC], BF16, tag="wq_sb")
    nc.sync.dma_start(wq_sb, d["wqT"])
    wk_sb = pool_w.tile([P, KX, FC], BF16, tag="wk_sb")
    nc.sync.dma_start(wk_sb, d["wkT"])
    wv_sb = pool_w.tile([P, KX, FC], BF16, tag="wv_sb")
    nc.sync.dma_start(wv_sb, d["wvT"])
    adjq_w = pool_w.tile([2, FC], F32R, tag="adjq_w")
    nc.sync.dma_start(adjq_w, d["adjq"])
    adjk_w = pool_w.tile([2, FC], F32R, tag="adjk_w")
    nc.sync.dma_start(adjk_w, d["adjk"])
    adjv_w = pool_w.tile([2, FC], F32R, tag="adjv_w")
    nc.sync.dma_start(adjv_w, d["adjv"])

    adjx = pool_adj.tile([2, Q], F32R, tag="adjx")      # [mean ; std] rows
    adjz = pool_adj.tile([2, S], F32R, tag="adjz")
    rxB = pool_bc.tile([P, Q], F32, tag="rxB")          # 1/std broadcast
    rzB = pool_bc.tile([P, S], F32, tag="rzB")

    def ln_stats(aT, T, adj, rB, ps_stats, scr=None):
        """Per 512-token chunk: LN stats -> adj=[mean;std] rows and a
        [P, T] broadcast of 1/std (via gpsimd partition_broadcast)."""
        for ch in range(T // 512):
            sl = slice(ch * 512, (ch + 1) * 512)
            ps_sum = ps_stats.tile([1, 512], F32, name="ps_sum", tag="ps_sum")
            ps_ssq = ps_stats.tile([1, 512], F32, name="ps_ssq", tag="ps_ssq")
            for k in range(KX):
                nc.tensor.matmul(ps_sum, ones_bcol, aT[:, k, sl],
                                 start=(k == 0), stop=(k == KX - 1))
                sq = pool_sq.tile([P, 512], BF16, name="sq", tag="sq")
                nc.scalar.square(sq, aT[:, k, sl])
                nc.tensor.matmul(ps_ssq, ones_bcol, sq,
                                 start=(k == 0), stop=(k == KX - 1))
            e2 = pool_rows.tile([1, 512], F32, name="e2", tag="e2")
            m2 = pool_rows.tile([1, 512], F32, name="m2", tag="m2")
            inv = pool_rows.tile([1, 512], F32R, name="inv", tag="inv")
            rr = pool_rows.tile([1, 512], F32R, name="rr", tag="rr")
            nc.vector.tensor_scalar_mul(adj[0:1, sl], ps_sum, 1.0 / D)  # mean
            nc.vector.tensor_scalar_mul(e2, ps_ssq, 1.0 / D)            # E[x^2]
            nc.vector.tensor_mul(m2, adj[0:1, sl], adj[0:1, sl])
            nc.vector.tensor_sub(e2, e2, m2)                            # var
            nc.scalar.activation(inv, e2, AF.Sqrt, bias=eps_t[0:1])     # std
            nc.vector.reciprocal_approx_fast(_f(rr), _f(inv))
            nc.sync.dma_start(adj[1:2, sl], inv)   # cross-partition row move
            nc.gpsimd.partition_broadcast(rB[:, sl], _f(rr))
            if scr is not None:
                nc.sync.dma_start(scr[0:1, sl], rr)

    # ---- x statistics + q projection ----
    with tc.tile_pool(name="ps_sx", bufs=2, space="PSUM") as ps_sx:
        ln_stats(xT, Q, adjx, rxB, ps_sx)

    with tc.tile_pool(name="ps_q", bufs=3, space="PSUM") as ps_qk:
        for m in range(2):
            for ch in range(2):
                sl = slice(ch * 512, (ch + 1) * 512)
                ps = ps_qk.tile([P, 512], F32, name="ps_qk_t", tag="ps_qk_t")
                for k in range(KX):
                    nc.tensor.matmul(ps, wq_sb[:, k, m * P:(m + 1) * P],
                                     xT[:, k, sl], start=(k == 0), stop=False)
                nc.tensor.matmul(ps, adjq_w[:, m * P:(m + 1) * P],
                                 _r(adjx[:, sl]), start=False, stop=True)
                nc.vector.tensor_mul(qT[:, m, sl], ps, rxB[:, sl])

    # ---- z statistics + k/v projections ----
    with tc.tile_pool(name="ps_sz", bufs=2, space="PSUM") as ps_sz:
        ln_stats(zT, S, adjz, rzB, ps_sz, scr=rs_scr)
    nc.sync.dma_start(rz_col, rs_scr.rearrange("a (i p) -> (a p) i", p=P))

    # softmax-denominator ones column
    nc.sync.dma_start(
        v_sb[:, :, :, HD:HD + 1],
        d["cb"][:, P:P + 64].rearrange("p (a b c) -> p a b c", a=NST, c=1))

    with tc.tile_pool(name="ps_k", bufs=3, space="PSUM") as ps_qk, \
         tc.tile_pool(name="ps_v", bufs=2, space="PSUM") as ps_v:
        for m in range(2):
            for ch in range(4):
                sl = slice(ch * 512, (ch + 1) * 512)
                ps = ps_qk.tile([P, 512], F32, name="ps_qk_t", tag="ps_qk_t")
                for k in range(KX):
                    nc.tensor.matmul(ps, wk_sb[:, k, m * P:(m + 1) * P],
                                     zT[:, k, sl], start=(k == 0), stop=False)
                nc.tensor.matmul(ps, adjk_w[:, m * P:(m + 1) * P],
                                 _r(adjz[:, sl]), start=False, stop=True)
                nc.vector.tensor_mul(kT[:, m, sl], ps, rzB[:, sl])

        for t in range(NST):
            ps = ps_v.tile([P, FC], F32, name="ps_v_t", tag="ps_v_t")
            for k in range(KX):
                nc.tensor.matmul(ps, zT[:, k, t * P:(t + 1) * P],
                                 wv_sb[:, k, :], start=(k == 0), stop=False)
            nc.tensor.matmul(ps, _r(adjz[:, t * P:(t + 1) * P]), _r(adjv_w),
                             start=False, stop=True)
            nc.vector.tensor_scalar_mul(
                v_sb[:, t, :, 0:HD],
                ps.rearrange("p (h e) -> p h e", h=NH),
                _f(rz_col[:, t:t + 1]))

    pool_rows.close()
    pool_sq.close()
    pool_bc.close()
    pool_adj.close()
    pool_w.close()
    pool_z.close()
    pool_x.close()

    # ---------------- resident FFN weights (prefetched during attention) ----
    pool_w1 = _Pool(tc.tile_pool(name="w1r", bufs=1))
    w1sb = pool_w1.tile([P, FFP, KX, P], BF16, tag="w1sb")
    for j in range(FFP):
        nc.sync.dma_start(w1sb[:, j], d["w1p"][j])
    w2sb = pool_w1.tile([P, FFP, D], BF16, tag="w2sb")
    for j in range(FFP):
        nc.sync.dma_start(w2sb[:, j], d["w2T"][j * P:(j + 1) * P, :])

    # =================== attention (query-chunked) ===================
    pool_att2 = _Pool(tc.tile_pool(name="att2", bufs=2))
    pool_mk0 = _Pool(tc.tile_pool(name="mk0", bufs=6))
    pool_mk1 = _Pool(tc.tile_pool(name="mk1", bufs=6))
    pool_pr0 = _Pool(tc.tile_pool(name="pr0", bufs=3))
    pool_pr1 = _Pool(tc.tile_pool(name="pr1", bufs=3))
    pool_nrm = _Pool(tc.tile_pool(name="nrm", bufs=2))
    pool_osb = _Pool(tc.tile_pool(name="osb", bufs=3))

    for ci in range(2):
        qsl = slice(ci * QC, (ci + 1) * QC)
        att2 = [pool_att2.tile([P, QC], BF16, name=f"att2_{p}", tag=f"att2_{p}")
                for p in range(2)]

        ps_lg0_cm = tc.tile_pool(name="ps_lg0", bufs=2, space="PSUM")
        ps_lg1_cm = tc.tile_pool(name="ps_lg1", bufs=2, space="PSUM")
        ps_att_cm = tc.tile_pool(name="ps_att", bufs=1, space="PSUM")
        ps_lg0 = ps_lg0_cm.__enter__()
        ps_lg1 = ps_lg1_cm.__enter__()
        ps_att = ps_att_cm.__enter__()

        for pair in range(2):
            attps = [ps_att.tile([HD + 1, QC], F32, name=f"attps{hh}",
                                 tag=f"attps{hh}") for hh in range(2)]

            def emit_pv(st, p0, p1):
                nc.tensor.matmul(attps[0], v_sb[:, st, 2 * pair, :], p0,
                                 start=(st == 0), stop=(st == NST - 1))
                nc.tensor.matmul(attps[1], v_sb[:, st, 2 * pair + 1, :], p1,
                                 start=(st == 0), stop=(st == NST - 1))

            prev = None
            for st in range(NST):
                ssl = slice(st * P, (st + 1) * P)
                mk0 = pool_mk0.tile([P, QC], BF16, name="mk0", tag="mk0")
                nc.sync.dma_start(mk0, d["maskT"][ci, 2 * pair, ssl, :])
                mk1 = pool_mk1.tile([P, QC], BF16, name="mk1", tag="mk1")
                nc.sync.dma_start(mk1, d["maskT"][ci, 2 * pair + 1, ssl, :])
                lg0 = ps_lg0.tile([P, QC], F32, name="lg0", tag="lg0")
                lg1 = ps_lg1.tile([P, QC], F32, name="lg1", tag="lg1")
                # head-pair QK in two concurrent 64-row PE groups
                nc.tensor.matmul(lg0, kT[0:HD, pair, ssl], qT[0:HD, pair, qsl],
                                 start=True, stop=False)
                nc.tensor.matmul(lg1, kT[HD:P, pair, ssl], qT[HD:P, pair, qsl],
                                 start=True, stop=False)
                # mask add via identity matmul into the same PSUM group
                nc.tensor.matmul(lg0, ident, mk0, start=False, stop=True)
                nc.tensor.matmul(lg1, ident, mk1, start=False, stop=True)
                pr0 = pool_pr0.tile([P, QC], BF16, name="pr0", tag="pr0")
                nc.scalar.activation(pr0, lg0, AF.Exp)
                pr1 = pool_pr1.tile([P, QC], BF16, name="pr1", tag="pr1")
                nc.scalar.activation(pr1, lg1, AF.Exp)
                if prev is not None:
                    emit_pv(*prev)
                prev = (st, pr0, pr1)
            emit_pv(*prev)

            # normalize: att2[pair][64h:64h+64] = attps[h][0:64] / attps[h][64]
            for hh in range(2):
                den = pool_nrm.tile([1, QC], F32, name="den", tag="den")
                nc.vector.tensor_copy(den, attps[hh][HD:HD + 1, :])
                r0 = pool_nrm.tile([1, QC], F32, name="r0", tag="r0")
                nc.vector.reciprocal_approx_fast(r0, den)
                nbc = pool_nrm.tile([HD, QC], F32, name="nbc", tag="nbc")
                nc.gpsimd.partition_broadcast(nbc, r0)
                nc.vector.tensor_mul(att2[pair][HD * hh:HD * hh + HD, :],
                                     attps[hh][0:HD, :], nbc)

        ps_att_cm.__exit__(None, None, None)
        ps_lg1_cm.__exit__(None, None, None)
        ps_lg0_cm.__exit__(None, None, None)

        # ---- out-projection for this chunk + ReduceScatter ----
        with tc.tile_pool(name="ps_o", bufs=2, space="PSUM") as ps_o:
            for m in range(KX):
                ps = ps_o.tile([P, QC], F32, name="ps_o_t", tag="ps_o_t")
                nc.tensor.matmul(ps, ow_sb[:, 0, m * P:(m + 1) * P], att2[0],
                                 start=True, stop=False)
                nc.tensor.matmul(ps, ow_sb[:, 1, m * P:(m + 1) * P], att2[1],
                                 start=False, stop=True)
                ot = pool_osb.tile([P, QC], BF16, name="ot", tag="ot")
                nc.vector.tensor_copy(ot, ps)
                for r2 in range(4):
                    nc.sync.dma_start(
                        rs_in[ci][r2, m * P:(m + 1) * P, :],
                        ot[:, r2 * P:(r2 + 1) * P])
        nc.gpsimd.collective_compute(
            "ReduceScatter",
            ALU.add,
            replica_groups=REPLICA_GROUPS,
            ins=[rs_in[ci].opt()],
            outs=[rs_out[ci].opt()],
        )

    pool_osb.close()
    pool_nrm.close()
    pool_pr1.close()
    pool_pr0.close()
    pool_mk1.close()
    pool_mk0.close()
    pool_att2.close()
    pool_qkv.close()

    # =================== residual + FFN (sequence-parallel) ===================
    pool_f = _Pool(tc.tile_pool(name="ffn", bufs=1, side="right"))
    pool_rsld = _Pool(tc.tile_pool(name="rsld", bufs=3))
    pool_fsq = _Pool(tc.tile_pool(name="fsq", bufs=2))
    pool_frow = _Pool(tc.tile_pool(name="frow", bufs=2))
    pool_ftmp = _Pool(tc.tile_pool(name="ftmp", bufs=2))
    pool_yo = _Pool(tc.tile_pool(name="yout", bufs=3))

    y1T = pool_f.tile([P, KX, QS], F32R, tag="y1T")
    y1n = pool_f.tile([P, KX, QS], BF16, tag="y1n")
    g_sb = pool_f.tile([P, FFP, QS], BF16, tag="g_sb")
    m_row = pool_f.tile([1, QS], F32, tag="m_row")
    r_row = pool_f.tile([1, QS], F32, tag="r_row")
    myB = pool_f.tile([P, QS], F32, tag="myB")
    ryB = pool_f.tile([P, QS], F32, tag="ryB")

    # y1 = RS(out-proj partials) + x_slice + out_b, piece by piece so piece A
    # (and its stats) runs while the chunk-B ReduceScatter is still in flight
    with tc.tile_pool(name="ps_yst", bufs=2, space="PSUM") as ps_yst:
        for piece in range(2):
            psl = slice(piece * P, (piece + 1) * P)
            for k in range(KX):
                rst = pool_rsld.tile([P, P], BF16, name="rst", tag="rst")
                nc.gpsimd.dma_start(rst, rs_out[piece][k * P:(k + 1) * P, :])
                nc.vector.scalar_tensor_tensor(
                    out=y1T[:, k, psl], in0=rst, scalar=outb_col[:, k:k + 1],
                    in1=xq_sb[:, k, psl], op0=ALU.add, op1=ALU.add)
            ps_sum = ps_yst.tile([1, P], F32, name="ps_sum2", tag="ps_sum2")
            ps_ssq = ps_yst.tile([1, P], F32, name="ps_ssq2", tag="ps_ssq2")
            for k in range(KX):
                nc.tensor.matmul(ps_sum, _r(ones_col), y1T[:, k, psl],
                                 start=(k == 0), stop=(k == KX - 1))
                sqy = pool_fsq.tile([P, P], F32R, name="sqy", tag="sqy")
                nc.scalar.square(sqy, y1T[:, k, psl])
                nc.tensor.matmul(ps_ssq, _r(ones_col), _r(sqy),
                                 start=(k == 0), stop=(k == KX - 1))
            e2 = pool_frow.tile([1, P], F32, name="e2y", tag="e2y")
            m2 = pool_frow.tile([1, P], F32, name="m2y", tag="m2y")
            inv = pool_frow.tile([1, P], F32, name="invy", tag="invy")
            nc.vector.tensor_scalar_mul(m_row[0:1, psl], ps_sum, 1.0 / D)
            nc.vector.tensor_scalar_mul(e2, ps_ssq, 1.0 / D)
            nc.vector.tensor_mul(m2, m_row[0:1, psl], m_row[0:1, psl])
            nc.vector.tensor_sub(e2, e2, m2)
            nc.scalar.activation(inv, e2, AF.Sqrt, bias=eps_t[0:1])
            nc.vector.reciprocal_approx_fast(r_row[0:1, psl], inv)

    nc.gpsimd.partition_broadcast(myB, m_row)
    nc.gpsimd.partition_broadcast(ryB, r_row)
    for k in range(KX):
        tmp = pool_ftmp.tile([P, QS], F32, name="tmpn", tag="tmpn")
        nc.vector.tensor_sub(tmp, y1T[:, k, :], myB)
        nc.vector.tensor_mul(y1n[:, k, :], tmp, ryB)

    # ff1 + gelu + ff2, interleaved per hidden block
    with tc.tile_pool(name="ps_f", bufs=2, space="PSUM") as ps_f, \
         tc.tile_pool(name="ps_y2", bufs=1, space="PSUM") as ps_y2:
        y2a = [ps_y2.tile([P, QS], F32, name=f"y2a_{i}", tag=f"y2a_{i}",
                          bufs=1) for i in range(4)]

        def emit_y2(mi, half):
            yt = pool_yo.tile([P, QS], F32, name="yt", tag="yt")
            nc.vector.scalar_tensor_tensor(
                out=yt, in0=y2a[mi - 4 * half],
                scalar=b2_col[:, mi:mi + 1],
                in1=y1T[:, mi, :], op0=ALU.add, op1=ALU.add)
            nc.sync.dma_start(d["out"][mi * P:(mi + 1) * P, :], yt)

        # pass 1: ff1 + gelu per hidden block, ff2 for the low 4 d-blocks
        for j in range(FFP):
            ps = ps_f.tile([P, QS], F32, name="ps_f_t", tag="ps_f_t")
            for k in range(KX):
                nc.tensor.matmul(ps, w1sb[:, j, k, :], y1n[:, k, :],
                                 start=(k == 0), stop=(k == KX - 1))
            nc.scalar.activation(g_sb[:, j, :], ps, AF.Gelu,
                                 bias=b1_col[:, j:j + 1])
            for mi in range(4):
                nc.tensor.matmul(y2a[mi], w2sb[:, j, mi * P:(mi + 1) * P],
                                 g_sb[:, j, :],
                                 start=(j == 0), stop=(j == FFP - 1))
        for mi in range(4):
            emit_y2(mi, 0)
        # pass 2: ff2 for the high 4 d-blocks (g_sb and w2 are resident)
        for j in range(FFP):
            for mi in range(4, KX):
                nc.tensor.matmul(y2a[mi - 4], w2sb[:, j, mi * P:(mi + 1) * P],
                                 g_sb[:, j, :],
                                 start=(j == 0), stop=(j == FFP - 1))
        for mi in range(4, KX):
            emit_y2(mi, 1)

    pool_yo.close()
    pool_ftmp.close()
    pool_frow.close()
    pool_fsq.close()
    pool_rsld.close()
    pool_f.close()
    pool_w1.close()
    const.close()
    dram.close()


def host_prep(inputs):
    """Fold layernorm gains/biases into (bf16) weights; build per-core shards."""
    f32 = np.float32
    bf = ml_dtypes.bfloat16
    x = np.asarray(inputs["x"], f32)
    z = np.asarray(inputs["z"], f32)
    mask = np.asarray(inputs["attn_mask"], f32)
    gq = np.asarray(inputs["gq"], np.float64)
    bq = np.asarray(inputs["bq"], np.float64)
    gkv = np.asarray(inputs["gkv"], np.float64)
    bkv = np.asarray(inputs["bkv"], np.float64)
    gff = np.asarray(inputs["gff"], np.float64)
    bff = np.asarray(inputs["bff"], np.float64)
    ipw = np.asarray(inputs["in_proj_w"], np.float64)
    ipb = np.asarray(inputs["in_proj_b"], np.float64)
    out_w = np.asarray(inputs["out_w"], f32)
    out_b = np.asarray(inputs["out_b"], f32)
    w1 = np.asarray(inputs["w1"], np.float64)
    b1 = np.asarray(inputs["b1"], np.float64)
    w2 = np.asarray(inputs["w2"], f32)
    b2 = np.asarray(inputs["b2"], f32)

    wq, wk, wv = ipw[:D], ipw[D:2 * D], ipw[2 * D:]
    pq, pk, pv = ipb[:D], ipb[D:2 * D], ipb[2 * D:]
    scale = 1.0 / np.sqrt(HD)
    wq2 = ((wq * gq[None, :]) * scale).astype(bf)
    pq2 = ((wq @ bq + pq) * scale).astype(f32)
    wk2 = (wk * gkv[None, :]).astype(bf)
    pk2 = (wk @ bkv + pk).astype(f32)
    wv2 = (wv * gkv[None, :]).astype(bf)
    pv2 = (wv @ bkv + pv).astype(f32)
    # rowsums of the *rounded* weights so the mean correction is consistent
    wq2r = wq2.astype(np.float64)
    wk2r = wk2.astype(np.float64)
    wv2r = wv2.astype(np.float64)

    w1b = (w1 * gff[None, :]).astype(bf)
    b12 = (w1b.astype(np.float64) @ bff + b1).astype(f32)
    b1c = np.ascontiguousarray(b12.reshape(FFP, P).T)
    w1T = np.ascontiguousarray(w1b.T)                              # (D, FF)
    w1p = np.ascontiguousarray(
        w1T.reshape(KX, P, FFP, P).transpose(2, 1, 0, 3))
    w2T = np.ascontiguousarray(w2.T.astype(bf))                    # (FF, D)

    def pack_kxf(wT):  # (D, FC) bf16 -> (P, D//P, FC)
        return np.ascontiguousarray(wT.reshape(KX, P, FC).transpose(1, 0, 2))

    cb = np.concatenate(
        [np.eye(P, dtype=f32), np.ones((P, 64), f32)], axis=1).astype(bf)

    in_maps = []
    for c in range(NCORES):
        b, hg = c // 4, c % 4
        fs = slice(FC * hg, FC * hg + FC)
        qidx = np.r_[P * hg:P * hg + P, 512 + P * hg:512 + P * hg + P]
        xTb = np.ascontiguousarray(x[b].T)                         # (D, Q)
        mk = mask[16 * b + NH * hg:16 * b + NH * hg + NH]          # (NH, Q, S)
        mkT = mk.transpose(0, 2, 1)                                # (NH, S, Q)
        maskT = np.ascontiguousarray(
            np.stack([mkT[:, :, 0:QC], mkT[:, :, QC:]], axis=0)).astype(bf)
        in_maps.append({
            "ones_t": np.ones((P, P), f32),
            "cb": cb,
            "xT": xTb.astype(bf),
            "zT": np.ascontiguousarray(z[b].T).astype(bf),
            "xq": np.ascontiguousarray(xTb[:, qidx]),
            "maskT": maskT,
            "wqT": pack_kxf(np.ascontiguousarray(wq2[fs].T)),
            "wkT": pack_kxf(np.ascontiguousarray(wk2[fs].T)),
            "wvT": pack_kxf(np.ascontiguousarray(wv2[fs].T)),
            "adjq": np.ascontiguousarray(
                np.stack([-wq2r[fs].sum(1), pq2[fs]]).astype(f32)),
            "adjk": np.ascontiguousarray(
                np.stack([-wk2r[fs].sum(1), pk2[fs]]).astype(f32)),
            "adjv": np.ascontiguousarray(
                np.stack([-wv2r[fs].sum(1), pv2[fs]]).astype(f32)),
            "owp": np.ascontiguousarray(
                out_w[:, fs].T.reshape(2, P, D).transpose(1, 0, 2)).astype(bf),
            "outb": out_b,
            "b1c": b1c,
            "b2": b2,
            "w1p": w1p,
            "w2T": w2T,
        })
    return in_maps


_NC_CACHE = None


def kernel(**inputs) -> np.ndarray:
    global _NC_CACHE, LAST_RESULT
    from concourse.bass_utils import run_bass_kernel_spmd

    in_maps = host_prep(inputs)
    if _NC_CACHE is None:
        _NC_CACHE = build_nc()
    res = run_bass_kernel_spmd(
        _NC_CACHE, in_maps, core_ids=list(range(NCORES)),
        trace=bool(os.environ.get("BASS_TRACE")),
    )
    LAST_RESULT = res
    out = np.empty((B, Q, D), np.float32)
    for c in range(NCORES):
        b, hg = c // 4, c % 4
        yT = res.results[c]["out"]                    # (D, QS)
        out[b, P * hg:P * hg + P, :] = yT[:, 0:P].T
        out[b, 512 + P * hg:512 + P * hg + P, :] = yT[:, P:2 * P].T
    return out


# revision 13
# speedup vs baseline: 1.4078x; 1.1630x over previous
# Bass/Tile TRN2 kernel for nn_BiasedCrossDecoderLayer (dense cross-attention
# transformer decoder layer), SPMD over 8 NeuronCores.
#
# Sharding: core c -> batch b = c//4, head-group hg = c%4 (4 of 16 heads).
# Attention is head-parallel and processed in two query chunks of 512; after
# each chunk the out-projection partial sums are ReduceScattered (bf16) within
# the 4-core batch group so the collective overlaps the next chunk's compute.
# Query ownership is interleaved: core c owns queries [128c,128c+128) of chunk
# A and [512+128c, 512+128c+128) of chunk B (the host gather restores order),
# so each chunk's RS delivers a 128-query piece directly.  The FFN then runs
# sequence-parallel on the core's 256 owned queries with the full 4096 hidden.
#
# LayerNorms are folded into the weights host-side (bf16 weights; rank-2
# [mean;std] correction matmuls in fp32r appended to each PSUM group).  The
# attention mask is added to the logits by an identity-matmul into the same
# PSUM accumulation group (no DVE pass); softmax denominators come from a
# ones-column appended to V (M=65 PV matmul).  The whole heavy datapath is
# bf16 (weights, activations, mask, probs) which halves DMA traffic and
# enables fast weight loads; PSUM accumulation stays fp32.  QK logits for a
# head pair run concurrently in two 64-row PE groups (K=64 row tiling).
#
# The FFN weights (w1 AND w2, bf16) are DMA'd into SBUF during attention so
# ff1/ff2 run back-to-back per hidden block with zero weight stalls.

import os
import sys

import numpy as np

sys.path.insert(0, "/opt/trn_rl_repo")

import ml_dtypes  # noqa: E402

import concourse.bass as bass  # noqa: E402
import concourse.mybir as mybir  # noqa: E402
import concourse.tile as tile  # noqa: E402
from concourse import bacc  # noqa: E402

F32 = mybir.dt.float32
F32R = mybir.dt.float32r
BF16 = mybir.dt.bfloat16
AF = mybir.ActivationFunctionType
ALU = mybir.AluOpType

B, Q, S, D, H = 2, 1024, 2048, 1024, 16
HD = D // H       # 64
FF = 4 * D
EPS = 1e-5
NCORES = 8
NH = 4            # heads per core
FC = NH * HD      # 256 qkv feature dims per core
QS = 256          # queries owned per core (two 128-query pieces)
QC = 512          # attention query chunk
P = 128
KX = D // P       # 8 k-tiles over the model dim
FFP = FF // P     # 32 hidden blocks
NST = S // P      # 16 s-tiles

REPLICA_GROUPS = [[0, 1, 2, 3], [4, 5, 6, 7]]

LAST_RESULT = None  # BassKernelResults of the most recent run (for test.py)


def _r(ap):
    return ap.bitcast(F32R)


def _f(ap):
    return ap.bitcast(F32)


def build_nc():
    nc = bacc.Bacc(
        "TRN2",
        target_bir_lowering=False,
        debug=False,
        num_devices=NCORES,
        name="biased_cross_decoder",
    )

    d = {}
    d["ones_t"] = nc.dram_tensor("ones_t", [P, P], F32R, kind="ExternalInput").ap()
    d["cb"] = nc.dram_tensor("cb", [P, P + 64], BF16, kind="ExternalInput").ap()
    d["xT"] = nc.dram_tensor("xT", [D, Q], BF16, kind="ExternalInput").ap()
    d["zT"] = nc.dram_tensor("zT", [D, S], BF16, kind="ExternalInput").ap()
    d["xq"] = nc.dram_tensor("xq", [D, QS], F32, kind="ExternalInput").ap()
    d["maskT"] = nc.dram_tensor("maskT", [2, 2, S, 2, QC], BF16,
                                kind="ExternalInput").ap()
    d["wqT"] = nc.dram_tensor("wqT", [P, KX, FC], BF16, kind="ExternalInput").ap()
    d["wkT"] = nc.dram_tensor("wkT", [P, KX, FC], BF16, kind="ExternalInput").ap()
    d["wvT"] = nc.dram_tensor("wvT", [P, KX, FC], BF16, kind="ExternalInput").ap()
    d["adjq"] = nc.dram_tensor("adjq", [2, FC], F32R, kind="ExternalInput").ap()
    d["adjk"] = nc.dram_tensor("adjk", [2, FC], F32R, kind="ExternalInput").ap()
    d["adjv"] = nc.dram_tensor("adjv", [2, FC], F32R, kind="ExternalInput").ap()
    d["owp"] = nc.dram_tensor("owp", [P, 2, D], BF16, kind="ExternalInput").ap()
    d["outb"] = nc.dram_tensor("outb", [D], F32, kind="ExternalInput").ap()
    d["b1c"] = nc.dram_tensor("b1c", [P, FFP], F32, kind="ExternalInput").ap()
    d["b2"] = nc.dram_tensor("b2", [D], F32, kind="ExternalInput").ap()
    d["w1p"] = nc.dram_tensor("w1p", [FFP, P, KX, P], BF16,
                              kind="ExternalInput").ap()
    d["w2T"] = nc.dram_tensor("w2T", [FF, D], BF16, kind="ExternalInput").ap()
    d["out"] = nc.dram_tensor("out", [D, QS], F32, kind="ExternalOutput").ap()

    with tile.TileContext(nc) as tc:
        build_tile_program(tc, nc, d)
    nc.compile()
    return nc


class _Pool:
    """Keeps the tile_pool context manager alive; allows explicit close."""

    def __init__(self, cm):
        self._cm = cm
        self.pool = cm.__enter__()

    def tile(self, *a, **kw):
        kw.setdefault("name", kw.get("tag") or "t")
        return self.pool.tile(*a, **kw)

    def close(self):
        self._cm.__exit__(None, None, None)


def build_tile_program(tc, nc, d):
    # ---------------- persistent constants ----------------
    const = _Pool(tc.tile_pool(name="const", bufs=1))
    dram = _Pool(tc.tile_pool(name="dram", bufs=1, space="DRAM"))

    ones_sb = const.tile([P, P], F32R, tag="ones_sb")
    nc.sync.dma_start(ones_sb, d["ones_t"])
    cb_sb = const.tile([P, P + 64], BF16, tag="cb_sb")
    nc.sync.dma_start(cb_sb, d["cb"])
    ident = cb_sb[:, 0:P]            # bf16 identity (mask-add matmul lhsT)
    ones_bcol = cb_sb[:, P:P + 1]    # bf16 ones column (bf16 stat sums)
    ones_col = ones_sb[:, 0:1]       # f32r ones column (fp32 stat sums)
    ones_row = ones_sb[0:1, :]       # f32r row (partition broadcasts)

    eps_t = const.tile([1, 1], F32, tag="eps")
    nc.vector.memset(eps_t, EPS)
    outb_col = const.tile([P, KX], F32, tag="outb_col")
    nc.sync.dma_start(outb_col, d["outb"].rearrange("(o p) -> p o", p=P))
    b2_col = const.tile([P, KX], F32, tag="b2_col")
    nc.sync.dma_start(b2_col, d["b2"].rearrange("(o p) -> p o", p=P))
    b1_col = const.tile([P, FFP], F32, tag="b1_col")
    nc.sync.dma_start(b1_col, d["b1c"])
    xq_sb = const.tile([P, KX, QS], F32, tag="xq_sb")
    nc.sync.dma_start(xq_sb, d["xq"].rearrange("(k p) q -> p k q", p=P))
    rz_col = const.tile([P, NST], F32R, tag="rz_col")

    rs_scr = dram.tile([1, S], F32R, tag="rs_scr")
    rs_in = [dram.tile([4, D, P], BF16, name=f"rs_in{i}", tag=f"rs_in{i}")
             for i in range(2)]
    rs_out = [dram.tile([D, P], BF16, name=f"rs_out{i}", tag=f"rs_out{i}")
              for i in range(2)]

    # ---------------- long-lived right-side pools ----------------
    pool_qkv = _Pool(tc.tile_pool(name="qkv", bufs=1, side="right"))
    qT = pool_qkv.tile([P, 2, Q], BF16, tag="qT")
    kT = pool_qkv.tile([P, 2, S], BF16, tag="kT")
    v_sb = pool_qkv.tile([P, NST, NH, HD + 1], BF16, tag="v_sb")
    ow_sb = pool_qkv.tile([P, 2, D], BF16, tag="ow_sb")
    nc.sync.dma_start(ow_sb, d["owp"])

    # ---------------- phase A scratch (left stack) ----------------
    pool_x = _Pool(tc.tile_pool(name="px", bufs=1))
    pool_z = _Pool(tc.tile_pool(name="pz", bufs=1))
    pool_w = _Pool(tc.tile_pool(name="pw", bufs=1))
    pool_adj = _Pool(tc.tile_pool(name="adj", bufs=1))
    pool_bc = _Pool(tc.tile_pool(name="bc", bufs=1))
    pool_sq = _Pool(tc.tile_pool(name="sq", bufs=3))
    pool_rows = _Pool(tc.tile_pool(name="rows", bufs=2))

    xT = pool_x.tile([P, KX, Q], BF16, tag="xT")
    for k in range(KX):
        nc.sync.dma_start(xT[:, k, :], d["xT"][k * P:(k + 1) * P, :])
    zT = pool_z.tile([P, KX, S], BF16, tag="zT")
    for h2 in range(2):
        for k in range(KX):
            nc.sync.dma_start(zT[:, k, h2 * 1024:(h2 + 1) * 1024],
                              d["zT"][k * P:(k + 1) * P, h2 * 1024:(h2 + 1) * 1024])

    wq_sb = pool_w.tile([P, KX, FC], BF16, tag="wq_sb")
    nc.sync.dma_start(wq_sb, d["wqT"])
    wk_sb = pool_w.tile([P, KX, FC], BF16, tag="wk_sb")
    nc.sync.dma_start(wk_sb, d["wkT"])
    wv_sb = pool_w.tile([P, KX, FC], BF16, tag="wv_sb")
    nc.sync.dma_start(wv_sb, d["wvT"])
    adjq_w = pool_w.tile([2, FC], F32R, tag="adjq_w")
    nc.sync.dma_start(adjq_w, d["adjq"])
    adjk_w = pool_w.tile([2, FC], F32R, tag="adjk_w")
    nc.sync.dma_start(adjk_w, d["adjk"])
    adjv_w = pool_w.tile([2, FC], F32R, tag="adjv_w")
    nc.sync.dma_start(adjv_w, d["adjv"])

    adjx = pool_adj.tile([2, Q], F32R, tag="adjx")      # [mean ; std] rows
    adjz = pool_adj.tile([2, S], F32R, tag="adjz")
    rxB = pool_bc.tile([P, Q], F32, tag="rxB")          # 1/std broadcast
    rzB = pool_bc.tile([P, S], F32, tag="rzB")

    def ln_stats(aT, T, adj, rB, ps_stats, scr=None):
        """Per 512-token chunk: LN stats -> adj=[mean;std] rows and a
        [P, T] broadcast of 1/std (via gpsimd partition_broadcast)."""
        for ch in range(T // 512):
            sl = slice(ch * 512, (ch + 1) * 512)
            ps_sum = ps_stats.tile([1, 512], F32, name="ps_sum", tag="ps_sum")
            ps_ssq = ps_stats.tile([1, 512], F32, name="ps_ssq", tag="ps_ssq")
            for k in range(KX):
                nc.tensor.matmul(ps_sum, ones_bcol, aT[:, k, sl],
                                 start=(k == 0), stop=(k == KX - 1))
                sq = pool_sq.tile([P, 512], BF16, name="sq", tag="sq")
                nc.scalar.square(sq, aT[:, k, sl])
                nc.tensor.matmul(ps_ssq, ones_bcol, sq,
                                 start=(k == 0), stop=(k == KX - 1))
            e2 = pool_rows.tile([1, 512], F32, name="e2", tag="e2")
            m2 = pool_rows.tile([1, 512], F32, name="m2", tag="m2")
            inv = pool_rows.tile([1, 512], F32R, name="inv", tag="inv")
            rr = pool_rows.tile([1, 512], F32R, name="rr", tag="rr")
            nc.vector.tensor_scalar_mul(adj[0:1, sl], ps_sum, 1.0 / D)  # mean
            nc.vector.tensor_scalar_mul(e2, ps_ssq, 1.0 / D)            # E[x^2]
            nc.vector.tensor_mul(m2, adj[0:1, sl], adj[0:1, sl])
            nc.vector.tensor_sub(e2, e2, m2)                            # var
            nc.scalar.activation(inv, e2, AF.Sqrt, bias=eps_t[0:1])     # std
            nc.vector.reciprocal_approx_fast(_f(rr), _f(inv))
            nc.scalar.dma_start(adj[1:2, sl], inv)   # cross-partition row move
            nc.gpsimd.partition_broadcast(rB[:, sl], _f(rr))
            if scr is not None:
                nc.scalar.dma_start(scr[0:1, sl], rr)

    # ---- x statistics + q projection ----
    with tc.tile_pool(name="ps_sx", bufs=2, space="PSUM") as ps_sx:
        ln_stats(xT, Q, adjx, rxB, ps_sx)

    with tc.tile_pool(name="ps_q", bufs=3, space="PSUM") as ps_qk:
        for m in range(2):
            for ch in range(2):
                sl = slice(ch * 512, (ch + 1) * 512)
                ps = ps_qk.tile([P, 512], F32, name="ps_qk_t", tag="ps_qk_t")
                for k in range(KX):
                    nc.tensor.matmul(ps, wq_sb[:, k, m * P:(m + 1) * P],
                                     xT[:, k, sl], start=(k == 0), stop=False)
                nc.tensor.matmul(ps, adjq_w[:, m * P:(m + 1) * P],
                                 _r(adjx[:, sl]), start=False, stop=True)
                nc.vector.tensor_mul(qT[:, m, sl], ps, rxB[:, sl])

    # ---- z statistics + k/v projections ----
    with tc.tile_pool(name="ps_sz", bufs=2, space="PSUM") as ps_sz:
        ln_stats(zT, S, adjz, rzB, ps_sz, scr=rs_scr)
    nc.scalar.dma_start(rz_col, rs_scr.rearrange("a (i p) -> (a p) i", p=P))

    # softmax-denominator ones column
    nc.sync.dma_start(
        v_sb[:, :, :, HD:HD + 1],
        d["cb"][:, P:P + 64].rearrange("p (a b c) -> p a b c", a=NST, c=1))

    with tc.tile_pool(name="ps_k", bufs=3, space="PSUM") as ps_qk, \
         tc.tile_pool(name="ps_v", bufs=2, space="PSUM") as ps_v:
        for m in range(2):
            for ch in range(4):
                sl = slice(ch * 512, (ch + 1) * 512)
                ps = ps_qk.tile([P, 512], F32, name="ps_qk_t", tag="ps_qk_t")
                for k in range(KX):
                    nc.tensor.matmul(ps, wk_sb[:, k, m * P:(m + 1) * P],
                                     zT[:, k, sl], start=(k == 0), stop=False)
                nc.tensor.matmul(ps, adjk_w[:, m * P:(m + 1) * P],
                                 _r(adjz[:, sl]), start=False, stop=True)
                nc.vector.tensor_mul(kT[:, m, sl], ps, rzB[:, sl])

        for t in range(NST):
            ps = ps_v.tile([P, FC], F32, name="ps_v_t", tag="ps_v_t")
            for k in range(KX):
                nc.tensor.matmul(ps, zT[:, k, t * P:(t + 1) * P],
                                 wv_sb[:, k, :], start=(k == 0), stop=False)
            nc.tensor.matmul(ps, _r(adjz[:, t * P:(t + 1) * P]), _r(adjv_w),
                             start=False, stop=True)
            nc.vector.tensor_scalar_mul(
                v_sb[:, t, :, 0:HD],
                ps.rearrange("p (h e) -> p h e", h=NH),
                _f(rz_col[:, t:t + 1]))

    pool_rows.close()
    pool_sq.close()
    pool_bc.close()
    pool_adj.close()
    pool_w.close()
    pool_z.close()
    pool_x.close()

    # ---------------- resident FFN weights (prefetched during attention) ----
    pool_w1 = _Pool(tc.tile_pool(name="w1r", bufs=1))
    w1sb = pool_w1.tile([P, FFP, KX, P], BF16, tag="w1sb")
    w2sb = pool_w1.tile([P, FFP, D], BF16, tag="w2sb")
    for j0 in range(0, FFP, 8):
        nc.gpsimd.dma_start(
            w1sb[:, j0:j0 + 8],
            d["w1p"][j0:j0 + 8].rearrange("j p k o -> p j k o"))
        nc.gpsimd.dma_start(
            w2sb[:, j0:j0 + 8],
            d["w2T"][j0 * P:(j0 + 8) * P, :].rearrange("(j p) o -> p j o", p=P))

    # =================== attention (query-chunked) ===================
    pool_att2 = _Pool(tc.tile_pool(name="att2", bufs=2))
    pool_mk = _Pool(tc.tile_pool(name="mk", bufs=6))
    pool_pr0 = _Pool(tc.tile_pool(name="pr0", bufs=3))
    pool_pr1 = _Pool(tc.tile_pool(name="pr1", bufs=3))
    pool_nrm = _Pool(tc.tile_pool(name="nrm", bufs=2))
    pool_osb = _Pool(tc.tile_pool(name="osb", bufs=3))

    for ci in range(2):
        qsl = slice(ci * QC, (ci + 1) * QC)
        att2 = [pool_att2.tile([P, QC], BF16, name=f"att2_{p}", tag=f"att2_{p}")
                for p in range(2)]

        ps_lg0_cm = tc.tile_pool(name="ps_lg0", bufs=2, space="PSUM")
        ps_lg1_cm = tc.tile_pool(name="ps_lg1", bufs=2, space="PSUM")
        ps_att_cm = tc.tile_pool(name="ps_att", bufs=1, space="PSUM")
        ps_lg0 = ps_lg0_cm.__enter__()
        ps_lg1 = ps_lg1_cm.__enter__()
        ps_att = ps_att_cm.__enter__()

        for pair in range(2):
            attps = [ps_att.tile([HD + 1, QC], F32, name=f"attps{hh}",
                                 tag=f"attps{hh}") for hh in range(2)]

            def emit_pv(st, p0, p1):
                nc.tensor.matmul(attps[0], v_sb[:, st, 2 * pair, :], p0,
                                 start=(st == 0), stop=(st == NST - 1))
                nc.tensor.matmul(attps[1], v_sb[:, st, 2 * pair + 1, :], p1,
                                 start=(st == 0), stop=(st == NST - 1))

            prev = None
            for st in range(NST):
                ssl = slice(st * P, (st + 1) * P)
                mk = pool_mk.tile([P, 2, QC], BF16, name="mk", tag="mk")
                nc.sync.dma_start(mk, d["maskT"][ci, pair, ssl, :, :])
                lg0 = ps_lg0.tile([P, QC], F32, name="lg0", tag="lg0")
                lg1 = ps_lg1.tile([P, QC], F32, name="lg1", tag="lg1")
                # head-pair QK in two concurrent 64-row PE groups
                nc.tensor.matmul(lg0, kT[0:HD, pair, ssl], qT[0:HD, pair, qsl],
                                 start=True, stop=False)
                nc.tensor.matmul(lg1, kT[HD:P, pair, ssl], qT[HD:P, pair, qsl],
                                 start=True, stop=False)
                # mask add via identity matmul into the same PSUM group
                nc.tensor.matmul(lg0, ident, mk[:, 0, :], start=False, stop=True)
                nc.tensor.matmul(lg1, ident, mk[:, 1, :], start=False, stop=True)
                pr0 = pool_pr0.tile([P, QC], BF16, name="pr0", tag="pr0")
                nc.scalar.activation(pr0, lg0, AF.Exp)
                pr1 = pool_pr1.tile([P, QC], BF16, name="pr1", tag="pr1")
                nc.scalar.activation(pr1, lg1, AF.Exp)
                if prev is not None:
                    emit_pv(*prev)
                prev = (st, pr0, pr1)
            emit_pv(*prev)

            # normalize: att2[pair][64h:64h+64] = attps[h][0:64] / attps[h][64]
            for hh in range(2):
                den = pool_nrm.tile([1, QC], F32, name="den", tag="den")
                nc.vector.tensor_copy(den, attps[hh][HD:HD + 1, :])
                r0 = pool_nrm.tile([1, QC], F32, name="r0", tag="r0")
                nc.vector.reciprocal_approx_fast(r0, den)
                nbc = pool_nrm.tile([HD, QC], F32, name="nbc", tag="nbc")
                nc.gpsimd.partition_broadcast(nbc, r0)
                nc.vector.tensor_mul(att2[pair][HD * hh:HD * hh + HD, :],
                                     attps[hh][0:HD, :], nbc)

        ps_att_cm.__exit__(None, None, None)
        ps_lg1_cm.__exit__(None, None, None)
        ps_lg0_cm.__exit__(None, None, None)

        # ---- out-projection for this chunk + ReduceScatter ----
        with tc.tile_pool(name="ps_o", bufs=2, space="PSUM") as ps_o:
            for m in range(KX):
                ps = ps_o.tile([P, QC], F32, name="ps_o_t", tag="ps_o_t")
                nc.tensor.matmul(ps, ow_sb[:, 0, m * P:(m + 1) * P], att2[0],
                                 start=True, stop=False)
                nc.tensor.matmul(ps, ow_sb[:, 1, m * P:(m + 1) * P], att2[1],
                                 start=False, stop=True)
                ot = pool_osb.tile([P, QC], BF16, name="ot", tag="ot")
                nc.vector.tensor_copy(ot, ps)
                nc.sync.dma_start(
                    rs_in[ci][:, m * P:(m + 1) * P, :].rearrange(
                        "r p q -> p r q"),
                    ot.rearrange("p (r q) -> p r q", r=4))
        nc.gpsimd.collective_compute(
            "ReduceScatter",
            ALU.add,
            replica_groups=REPLICA_GROUPS,
            ins=[rs_in[ci].opt()],
            outs=[rs_out[ci].opt()],
        )

    pool_osb.close()
    pool_nrm.close()
    pool_pr1.close()
    pool_pr0.close()
    pool_mk.close()
    pool_att2.close()
    pool_qkv.close()

    # =================== residual + FFN (sequence-parallel) ===================
    pool_f = _Pool(tc.tile_pool(name="ffn", bufs=1, side="right"))
    pool_rsld = _Pool(tc.tile_pool(name="rsld", bufs=3))
    pool_fsq = _Pool(tc.tile_pool(name="fsq", bufs=2))
    pool_frow = _Pool(tc.tile_pool(name="frow", bufs=2))
    pool_ftmp = _Pool(tc.tile_pool(name="ftmp", bufs=2))
    pool_yo = _Pool(tc.tile_pool(name="yout", bufs=3))

    y1T = pool_f.tile([P, KX, QS], F32R, tag="y1T")
    y1n = pool_f.tile([P, KX, QS], BF16, tag="y1n")
    g_sb = pool_f.tile([P, FFP, QS], BF16, tag="g_sb")
    m_row = pool_f.tile([1, QS], F32, tag="m_row")
    r_row = pool_f.tile([1, QS], F32, tag="r_row")
    myB = pool_f.tile([P, QS], F32, tag="myB")
    ryB = pool_f.tile([P, QS], F32, tag="ryB")

    # Full FFN per 128-query piece: piece A (chunk-A queries) runs while the
    # chunk-B ReduceScatter is still in flight; piece B follows after it lands.
    with tc.tile_pool(name="ps_yst", bufs=1, space="PSUM") as ps_yst, \
         tc.tile_pool(name="ps_f", bufs=2, space="PSUM") as ps_f, \
         tc.tile_pool(name="ps_y2", bufs=1, space="PSUM") as ps_y2:

        def emit_y2(y2a, mi, psl):
            yt = pool_yo.tile([P, P], F32, name="yt", tag="yt")
            nc.vector.scalar_tensor_tensor(
                out=yt, in0=y2a[mi % 4], scalar=b2_col[:, mi:mi + 1],
                in1=y1T[:, mi, psl], op0=ALU.add, op1=ALU.add)
            nc.sync.dma_start(d["out"][mi * P:(mi + 1) * P, psl], yt)

        for piece in range(2):
            psl = slice(piece * P, (piece + 1) * P)
            # y1 = RS(out-proj partials) + x_slice + out_b
            for k in range(KX):
                rst = pool_rsld.tile([P, P], BF16, name="rst", tag="rst")
                nc.gpsimd.dma_start(rst, rs_out[piece][k * P:(k + 1) * P, :])
                nc.vector.scalar_tensor_tensor(
                    out=y1T[:, k, psl], in0=rst, scalar=outb_col[:, k:k + 1],
                    in1=xq_sb[:, k, psl], op0=ALU.add, op1=ALU.add)
            # LN stats for the piece
            ps_sum = ps_yst.tile([1, P], F32, name="ps_sum2", tag="ps_sum2")
            ps_ssq = ps_yst.tile([1, P], F32, name="ps_ssq2", tag="ps_ssq2")
            for k in range(KX):
                nc.tensor.matmul(ps_sum, _r(ones_col), y1T[:, k, psl],
                                 start=(k == 0), stop=(k == KX - 1))
                sqy = pool_fsq.tile([P, P], F32R, name="sqy", tag="sqy")
                nc.scalar.square(sqy, y1T[:, k, psl])
                nc.tensor.matmul(ps_ssq, _r(ones_col), _r(sqy),
                                 start=(k == 0), stop=(k == KX - 1))
            e2 = pool_frow.tile([1, P], F32, name="e2y", tag="e2y")
            m2 = pool_frow.tile([1, P], F32, name="m2y", tag="m2y")
            inv = pool_frow.tile([1, P], F32, name="invy", tag="invy")
            nc.vector.tensor_scalar_mul(m_row[0:1, psl], ps_sum, 1.0 / D)
            nc.vector.tensor_scalar_mul(e2, ps_ssq, 1.0 / D)
            nc.vector.tensor_mul(m2, m_row[0:1, psl], m_row[0:1, psl])
            nc.vector.tensor_sub(e2, e2, m2)
            nc.scalar.activation(inv, e2, AF.Sqrt, bias=eps_t[0:1])
            nc.vector.reciprocal_approx_fast(r_row[0:1, psl], inv)
            nc.gpsimd.partition_broadcast(myB[:, psl], m_row[0:1, psl])
            nc.gpsimd.partition_broadcast(ryB[:, psl], r_row[0:1, psl])
            for k in range(KX):
                tmp = pool_ftmp.tile([P, P], F32, name="tmpn", tag="tmpn")
                nc.vector.tensor_sub(tmp, y1T[:, k, psl], myB[:, psl])
                nc.vector.tensor_mul(y1n[:, k, psl], tmp, ryB[:, psl])

            # ff1 + gelu per hidden block, ff2 pass 1 (low 4 d-blocks)
            y2a = [ps_y2.tile([P, P], F32, name=f"y2a_{i}", tag=f"y2a_{i}")
                   for i in range(4)]
            for j in range(FFP):
                ps = ps_f.tile([P, P], F32, name="ps_f_t", tag="ps_f_t")
                for k in range(KX):
                    nc.tensor.matmul(ps, w1sb[:, j, k, :], y1n[:, k, psl],
                                     start=(k == 0), stop=(k == KX - 1))
                nc.scalar.activation(g_sb[:, j, psl], ps, AF.Gelu,
                                     bias=b1_col[:, j:j + 1])
                for mi in range(4):
                    nc.tensor.matmul(y2a[mi], w2sb[:, j, mi * P:(mi + 1) * P],
                                     g_sb[:, j, psl],
                                     start=(j == 0), stop=(j == FFP - 1))
            for mi in range(4):
                emit_y2(y2a, mi, psl)
            # ff2 pass 2 (high 4 d-blocks) over the resident activations
            y2a = [ps_y2.tile([P, P], F32, name=f"y2a_{i}", tag=f"y2a_{i}")
                   for i in range(4)]
            for j in range(FFP):
                for mi in range(4, KX):
                    nc.tensor.matmul(y2a[mi - 4],
                                     w2sb[:, j, mi * P:(mi + 1) * P],
                                     g_sb[:, j, psl],
                                     start=(j == 0), stop=(j == FFP - 1))
            for mi in range(4, KX):
                emit_y2(y2a, mi, psl)

    pool_yo.close()
    pool_ftmp.close()
    pool_frow.close()
    pool_fsq.close()
    pool_rsld.close()
    pool_f.close()
    pool_w1.close()
    const.close()
    dram.close()


def host_prep(inputs):
    """Fold layernorm gains/biases into (bf16) weights; build per-core shards."""
    f32 = np.float32
    bf = ml_dtypes.bfloat16
    x = np.asarray(inputs["x"], f32)
    z = np.asarray(inputs["z"], f32)
    mask = np.asarray(inputs["attn_mask"], f32)
    gq = np.asarray(inputs["gq"], np.float64)
    bq = np.asarray(inputs["bq"], np.float64)
    gkv = np.asarray(inputs["gkv"], np.float64)
    bkv = np.asarray(inputs["bkv"], np.float64)
    gff = np.asarray(inputs["gff"], np.float64)
    bff = np.asarray(inputs["bff"], np.float64)
    ipw = np.asarray(inputs["in_proj_w"], np.float64)
    ipb = np.asarray(inputs["in_proj_b"], np.float64)
    out_w = np.asarray(inputs["out_w"], f32)
    out_b = np.asarray(inputs["out_b"], f32)
    w1 = np.asarray(inputs["w1"], np.float64)
    b1 = np.asarray(inputs["b1"], np.float64)
    w2 = np.asarray(inputs["w2"], f32)
    b2 = np.asarray(inputs["b2"], f32)

    wq, wk, wv = ipw[:D], ipw[D:2 * D], ipw[2 * D:]
    pq, pk, pv = ipb[:D], ipb[D:2 * D], ipb[2 * D:]
    scale = 1.0 / np.sqrt(HD)
    wq2 = ((wq * gq[None, :]) * scale).astype(bf)
    pq2 = ((wq @ bq + pq) * scale).astype(f32)
    wk2 = (wk * gkv[None, :]).astype(bf)
    pk2 = (wk @ bkv + pk).astype(f32)
    wv2 = (wv * gkv[None, :]).astype(bf)
    pv2 = (wv @ bkv + pv).astype(f32)
    # rowsums of the *rounded* weights so the mean correction is consistent
    wq2r = wq2.astype(np.float64)
    wk2r = wk2.astype(np.float64)
    wv2r = wv2.astype(np.float64)

    w1b = (w1 * gff[None, :]).astype(bf)
    b12 = (w1b.astype(np.float64) @ bff + b1).astype(f32)
    b1c = np.ascontiguousarray(b12.reshape(FFP, P).T)
    w1T = np.ascontiguousarray(w1b.T)                              # (D, FF)
    w1p = np.ascontiguousarray(
        w1T.reshape(KX, P, FFP, P).transpose(2, 1, 0, 3))
    w2T = np.ascontiguousarray(w2.T.astype(bf))                    # (FF, D)

    def pack_kxf(wT):  # (D, FC) bf16 -> (P, D//P, FC)
        return np.ascontiguousarray(wT.reshape(KX, P, FC).transpose(1, 0, 2))

    cb = np.concatenate(
        [np.eye(P, dtype=f32), np.ones((P, 64), f32)], axis=1).astype(bf)

    in_maps = []
    for c in range(NCORES):
        b, hg = c // 4, c % 4
        fs = slice(FC * hg, FC * hg + FC)
        qidx = np.r_[P * hg:P * hg + P, 512 + P * hg:512 + P * hg + P]
        xTb = np.ascontiguousarray(x[b].T)                         # (D, Q)
        mk = mask[16 * b + NH * hg:16 * b + NH * hg + NH]          # (NH, Q, S)
        mkT = mk.transpose(0, 2, 1)                                # (NH, S, Q)
        m5 = mkT.reshape(2, 2, S, Q).transpose(0, 2, 1, 3)         # (pair,S,hh,Q)
        maskT = np.ascontiguousarray(
            np.stack([m5[:, :, :, 0:QC], m5[:, :, :, QC:]],
                     axis=0)).astype(bf)                           # (ci,pair,S,hh,QC)
        in_maps.append({
            "ones_t": np.ones((P, P), f32),
            "cb": cb,
            "xT": xTb.astype(bf),
            "zT": np.ascontiguousarray(z[b].T).astype(bf),
            "xq": np.ascontiguousarray(xTb[:, qidx]),
            "maskT": maskT,
            "wqT": pack_kxf(np.ascontiguousarray(wq2[fs].T)),
            "wkT": pack_kxf(np.ascontiguousarray(wk2[fs].T)),
            "wvT": pack_kxf(np.ascontiguousarray(wv2[fs].T)),
            "adjq": np.ascontiguousarray(
                np.stack([-wq2r[fs].sum(1), pq2[fs]]).astype(f32)),
            "adjk": np.ascontiguousarray(
                np.stack([-wk2r[fs].sum(1), pk2[fs]]).astype(f32)),
            "adjv": np.ascontiguousarray(
                np.stack([-wv2r[fs].sum(1), pv2[fs]]).astype(f32)),
            "owp": np.ascontiguousarray(
                out_w[:, fs].T.reshape(2, P, D).transpose(1, 0, 2)).astype(bf),
            "outb": out_b,
            "b1c": b1c,
            "b2": b2,
            "w1p": w1p,
            "w2T": w2T,
        })
    return in_maps


_NC_CACHE = None


def kernel(**inputs) -> np.ndarray:
    global _NC_CACHE, LAST_RESULT
    from concourse.bass_utils import run_bass_kernel_spmd

    in_maps = host_prep(inputs)
    if _NC_CACHE is None:
        _NC_CACHE = build_nc()
    res = run_bass_kernel_spmd(
        _NC_CACHE, in_maps, core_ids=list(range(NCORES)),
        trace=bool(os.environ.get("BASS_TRACE")),
    )
    LAST_RESULT = res
    out = np.empty((B, Q, D), np.float32)
    for c in range(NCORES):
        b, hg = c // 4, c % 4
        yT = res.results[c]["out"]                    # (D, QS)
        out[b, P * hg:P * hg + P, :] = yT[:, 0:P].T
        out[b, 512 + P * hg:512 + P * hg + P, :] = yT[:, P:2 * P].T
    return out
